# revision 6
# baseline (speedup 1.0000x reference)
"""Multi-head attention (QKV proj + per-head RMSNorm + softmax attention +
output proj) for Trainium2, distributed over 8 NeuronCores.

Sharding: batch (2) x head-groups (4 heads per core), as one software-pipelined
pass (no phase barrier) so TensorE projection/output work overlaps the
ACT/DVE exp work of the attention phase.

Key structure per core (b = core//4, 4 heads = 2 pairs):
- X^T loaded pretransposed [D, T]; Q^T/K^T projections per pair in a packed
  [128 = 2 heads x 64d, T] layout (head A rows 0:64, head B rows 64:128).
- V projected directly in [token, d] layout (stationary = X^T tile, moving =
  Wv) into [V|1] staging tiles, so no PE transposes are needed and the 65th
  column accumulates softmax denominators during the AV matmul.
- RMSNorm: sum-of-squares via a [128,2] ones matmul, rstd = exp(-0.5*ln(.))
  on ACT in packed [8,512] tiles (thin [2,512] chunks are repacked via tiny
  DMAs), broadcast back via DRAM round-trip DMAs.  The softmax 1/8 scale and
  k_norm weight fold into khat; q_norm weight folds into qhat.
- Scores S^T = K^T.T @ Q^T run as TWO CONCURRENT row-tiled matmuls
  (tile_position (0,0) / (64,0), K=64 each) writing the two halves of one
  [128, 1024] PSUM tile -> 2x score throughput vs zero-padded K=128.
- exp over scores is split between ScalarE (table exp, bf16 out) and VectorE
  (Schraudolph bit-trick: i16 = s*A + B, bitcast to bf16) on a tunable
  fraction of key-tiles, so neither engine gates the TensorE pipeline.
- Attention output O^T accumulates in PSUM [65, 512] per (head, query-chunk);
  row 64 is the denominator.  Evictions go bf16 to SBUF staging, then DMA
  assembles ohp [128, T] per pair and a packed dn [16, 512] tile; ONE DVE
  reciprocal + DRAM broadcast gives 1/dn; output projection is a plain
  2-pair accumulated matmul, partial outputs summed on host.
"""

import os
import sys

for _p in ("/opt/trn_rl_repo",):
    if _p not in sys.path:
        sys.path.insert(0, _p)

import numpy as np

B = 2
T = 2048
D = 1024
H = 16
HD = 64
HPC = 4          # heads per core
NPAIR = 2        # head pairs per core
N_CORES = 8
EPS = 1e-5
TT = T // 128    # 16 key tiles
CT = D // 128    # 8 contraction tiles
QC = T // 512    # 4 query chunks

# Schraudolph bf16 exp: bits = trunc(s * A + B) as int16, bitcast to bf16.
SCHRA_A = 184.66496280360212     # 128 / ln(2)
SCHRA_B = 16249.05               # (127 - 6.95) * 128 rounded; tuned offline

_COMPILED = None
LAST_EXEC_NS = None


def _dve_exp_tile(pair, qc, kt):
    """Which (pair, qc, kt) exp tiles run on VectorE (Schraudolph)."""
    if pair == 0:
        return kt % 8 == 7
    return kt % 2 == 1


def _install_ntff_shim():
    """antenv.axon_hooks is missing in this image; provide it so that
    BASS_TRACE=1 profiling works (mirrors trn_boot's ctypes hook)."""
    import contextlib
    import ctypes
    import types

    if "antenv.axon_hooks" in sys.modules:
        return
    so_path = "/opt/axon/libaxon_pjrt.so"
    if not os.path.exists(so_path):
        return
    lib = ctypes.CDLL(so_path)
    if not hasattr(lib, "axon_start_nrt_profile"):
        return
    lib.axon_start_nrt_profile.argtypes = [ctypes.POINTER(ctypes.c_int64), ctypes.c_size_t]
    lib.axon_start_nrt_profile.restype = ctypes.c_int64
    lib.axon_stop_nrt_profile.argtypes = [ctypes.c_char_p]
    lib.axon_stop_nrt_profile.restype = ctypes.c_int64

    @contextlib.contextmanager
    def _hook(output_dir, device_ids):
        import jax

        jax.devices()
        if device_ids:
            ids = (ctypes.c_int64 * len(device_ids))(*device_ids)
            rc = lib.axon_start_nrt_profile(ids, len(device_ids))
        else:
            rc = lib.axon_start_nrt_profile(None, 0)
        if rc != 0:
            raise RuntimeError(f"axon_start_nrt_profile rc={rc}")
        try:
            yield
        finally:
            n = lib.axon_stop_nrt_profile(str(output_dir).encode())
            print(f"profile: {n} file(s) written to {output_dir}", file=sys.stderr)

    mod = types.ModuleType("antenv.axon_hooks")
    mod._hook = _hook
    mod.get_axon_ntff_profile_hook = lambda: mod._hook
    mod.set_axon_ntff_profile_hook = lambda h: setattr(mod, "_hook", h)
    sys.modules["antenv.axon_hooks"] = mod
    try:
        import antenv

        antenv.axon_hooks = mod
    except ImportError:
        pass


def _build():
    import concourse.bass as bass
    import concourse.tile as tile
    from concourse import bacc, mybir
    from contextlib import ExitStack

    F32 = mybir.dt.float32
    BF16 = mybir.dt.bfloat16
    I16 = mybir.dt.int16
    Exp = mybir.ActivationFunctionType.Exp
    Square = mybir.ActivationFunctionType.Square
    Log = mybir.ActivationFunctionType.Ln if hasattr(
        mybir.ActivationFunctionType, "Ln") else mybir.ActivationFunctionType.Log
    mult = mybir.AluOpType.mult
    add = mybir.AluOpType.add
    LN8 = float(np.log(0.125))

    nc = bacc.Bacc("TRN2", target_bir_lowering=False, debug=False, num_devices=N_CORES)

    xbT_d = nc.dram_tensor("xbT", (D, T), BF16, kind="ExternalInput").ap()
    wq_d = nc.dram_tensor("wq_s", (D, HPC * HD), BF16, kind="ExternalInput").ap()
    wk_d = nc.dram_tensor("wk_s", (D, HPC * HD), BF16, kind="ExternalInput").ap()
    wv_d = nc.dram_tensor("wv_s", (D, HPC * HD), BF16, kind="ExternalInput").ap()
    wo_d = nc.dram_tensor("wo_s", (HPC * HD, D), BF16, kind="ExternalInput").ap()
    bd2_d = nc.dram_tensor("bd2", (128, 2), BF16, kind="ExternalInput").ap()
    wqc_d = nc.dram_tensor("wqc", (128, 1), F32, kind="ExternalInput").ap()
    wkc_d = nc.dram_tensor("wkc", (128, 1), F32, kind="ExternalInput").ap()
    outT = nc.dram_tensor("outT", (D, T), BF16, kind="ExternalOutput").ap()

    with tile.TileContext(nc) as tc:
        with ExitStack() as top:
            consts = top.enter_context(tc.tile_pool(name="consts", bufs=1))
            xtp = top.enter_context(tc.tile_pool(name="xT", bufs=1))
            wpool = top.enter_context(tc.tile_pool(name="w", bufs=1))
            qkpool = top.enter_context(tc.tile_pool(name="qk", bufs=1))
            vppool = top.enter_context(tc.tile_pool(name="vp", bufs=1))
            stage = top.enter_context(tc.tile_pool(name="stage", bufs=3))
            sqp = top.enter_context(tc.tile_pool(name="sq", bufs=2))
            mstp = top.enter_context(tc.tile_pool(name="mst", bufs=2))
            msp = top.enter_context(tc.tile_pool(name="ms", bufs=1))
            rwp = top.enter_context(tc.tile_pool(name="rw", bufs=2))
            psb = top.enter_context(tc.tile_pool(name="p", bufs=3))
            psbi = top.enter_context(tc.tile_pool(name="pi", bufs=2))
            stgp = top.enter_context(tc.tile_pool(name="stg", bufs=3))
            ohpp = top.enter_context(tc.tile_pool(name="ohp", bufs=1))
            dnp = top.enter_context(tc.tile_pool(name="dn", bufs=1))
            rbp = top.enter_context(tc.tile_pool(name="rb", bufs=1))
            ohrp = top.enter_context(tc.tile_pool(name="ohr", bufs=1))
            osbp = top.enter_context(tc.tile_pool(name="osb", bufs=3))
            drp = top.enter_context(tc.tile_pool(name="dr", bufs=1, space="DRAM"))
            pp_score = top.enter_context(
                tc.tile_pool(name="ppscore", bufs=2, space="PSUM"))
            pp_o = top.enter_context(tc.tile_pool(name="ppo", bufs=2, space="PSUM"))
            pp_proj = top.enter_context(
                tc.tile_pool(name="ppproj", bufs=2, space="PSUM"))

            # ---- consts + weight/activation loads ----
            bd2 = consts.tile([128, 2], BF16, tag="bd2")
            nc.sync.dma_start(out=bd2[:], in_=bd2_d)
            wqc = consts.tile([128, 1], F32, tag="wqc")
            nc.sync.dma_start(out=wqc[:], in_=wqc_d)
            wkc = consts.tile([128, 1], F32, tag="wkc")
            nc.sync.dma_start(out=wkc[:], in_=wkc_d)
            epsc = consts.tile([128, 1], F32, tag="epsc")
            nc.vector.memset(epsc[:], EPS)
            ln8c = consts.tile([128, 1], F32, tag="ln8c")
            nc.vector.memset(ln8c[:], LN8)

            xT = []
            for c in range(CT):
                t = xtp.tile([128, T], BF16, tag=f"xT{c}", name=f"xT{c}")
                nc.sync.dma_start(out=t[:], in_=xbT_d[c * 128:(c + 1) * 128, :])
                xT.append(t)

            def load_w(dram, nm):
                ws = []
                for c in range(CT):
                    t = wpool.tile([128, HPC * HD], BF16, tag=f"w{nm}{c}",
                                   name=f"w{nm}{c}")
                    nc.sync.dma_start(out=t[:], in_=dram[c * 128:(c + 1) * 128, :])
                    ws.append(t)
                return ws

            wq = load_w(wq_d, "q")
            wk = load_w(wk_d, "k")
            wv = load_w(wv_d, "v")
            wo_sb = []
            for p in range(NPAIR):
                t = wpool.tile([128, D], BF16, tag=f"wo{p}", name=f"wo{p}")
                nc.sync.dma_start(out=t[:], in_=wo_d[p * 128:(p + 1) * 128, :])
                wo_sb.append(t)

            # persistent data tiles
            vp = [vppool.tile([128, TT, 2, 65], BF16, tag=f"vp{p}", name=f"vp{p}")
                  for p in range(NPAIR)]
            for p in range(NPAIR):
                nc.vector.memset(vp[p][:, :, :, 64:65], 1.0)
            qhat = [qkpool.tile([128, T], BF16, tag=f"qh{p}", name=f"qhat{p}")
                    for p in range(NPAIR)]
            khat = [qkpool.tile([128, T], BF16, tag=f"kh{p}", name=f"khat{p}")
                    for p in range(NPAIR)]
            ohp = [ohpp.tile([128, T], BF16, tag=f"ohp{p}", name=f"ohp{p}")
                   for p in range(NPAIR)]
            dn16 = dnp.tile([16, 512], BF16, tag="dn16", name="dn16")
            dnr16 = dnp.tile([16, 512], BF16, tag="dnr16", name="dnr16")
            rb = [rbp.tile([128, T], BF16, tag=f"rb{p}", name=f"rb{p}")
                  for p in range(NPAIR)]
            ohr = [ohrp.tile([128, T], BF16, tag=f"ohr{p}", name=f"ohr{p}")
                   for p in range(NPAIR)]

            # ---- V: direct [token, d] projection into [V|1] staging ----
            def vproj():
                for kt in range(TT):
                    pv = pp_proj.tile([128, HPC * HD], F32, tag="proj")
                    for ct in range(CT):
                        nc.tensor.matmul(
                            pv[:], xT[ct][:, kt * 128:(kt + 1) * 128], wv[ct][:],
                            start=(ct == 0), stop=(ct == CT - 1))
                    for pair in range(NPAIR):
                        src = pv[:, pair * 128:(pair + 1) * 128].rearrange(
                            "p (h d) -> p h d", h=2)
                        with nc.allow_low_precision(reason="bf16 v staging"):
                            if (kt + pair) % 2 == 0:
                                nc.scalar.copy(vp[pair][:, kt, :, 0:64], src)
                            else:
                                nc.vector.tensor_copy(
                                    out=vp[pair][:, kt, :, 0:64], in_=src)

            # ---- Q/K projection + RMSNorm chain for one (pair, kind) ----
            def proj_norm(pair, wt, wcol, dest, kind):
                stg = stage.tile([128, T], BF16, tag="qkstage",
                                 name=f"stg{kind}{pair}")
                ms8 = msp.tile([8, 512], F32, tag=f"ms{kind}{pair}",
                               name=f"ms{kind}{pair}")
                for qc in range(QC):
                    pq = pp_proj.tile([128, 512], F32, tag="proj")
                    for ct in range(CT):
                        nc.tensor.matmul(
                            pq[:], wt[ct][:, pair * 128:(pair + 1) * 128],
                            xT[ct][:, qc * 512:(qc + 1) * 512],
                            start=(ct == 0), stop=(ct == CT - 1))
                    sl = slice(qc * 512, (qc + 1) * 512)
                    with nc.allow_low_precision(reason="bf16 evict"):
                        nc.vector.tensor_scalar(stg[:, sl], pq[:], wcol[:], None,
                                                mult)
                    q2 = sqp.tile([128, 512], BF16, tag="q2")
                    with nc.allow_low_precision(reason="bf16 squares"):
                        nc.scalar.activation(q2[:], pq[:], Square)
                    msps = pp_proj.tile([128, 512], F32, tag="proj")
                    nc.tensor.matmul(msps[0:2, :], bd2[:], q2[:],
                                     start=True, stop=True)
                    mstmp = mstp.tile([2, 512], F32, tag="mstmp")
                    nc.vector.tensor_copy(out=mstmp[:], in_=msps[0:2, :])
                    nc.gpsimd.dma_start(out=ms8[2 * qc:2 * qc + 2, :], in_=mstmp[:])
                ms8ln = msp.tile([8, 512], F32, tag=f"msln{kind}{pair}",
                                 name=f"msln{kind}{pair}")
                nc.scalar.activation(ms8ln[:], ms8[:], Log,
                                     scale=1.0 / HD, bias=epsc[0:8, :])
                r8 = msp.tile([8, 512], BF16, tag=f"r8{kind}{pair}",
                              name=f"r8{kind}{pair}")
                with nc.allow_low_precision(reason="bf16 rstd"):
                    if kind == "k":
                        nc.scalar.activation(r8[:], ms8ln[:], Exp, scale=-0.5,
                                             bias=ln8c[0:8, :])
                    else:
                        nc.scalar.activation(r8[:], ms8ln[:], Exp, scale=-0.5)
                r8d = drp.tile([8, 512], BF16, tag=f"r8d{kind}{pair}",
                               name=f"r8d{kind}{pair}")
                nc.gpsimd.dma_start(out=r8d[:], in_=r8[:])
                rw = rwp.tile([128, T], BF16, tag="rw")
                for h in range(2):
                    for qc in range(QC):
                        row = r8d[2 * qc + h:2 * qc + h + 1, :]
                        brd = bass.AP(tensor=row.tensor, offset=row.offset,
                                      ap=[[0, 64]] + list(row.ap[1:]))
                        nc.gpsimd.dma_start(
                            out=rw[64 * h:64 * h + 64, qc * 512:(qc + 1) * 512],
                            in_=brd)
                with nc.allow_low_precision(reason="bf16 norm"):
                    nc.vector.tensor_mul(dest[:], stg[:], rw[:])

            # ---- attention for one pair ----
            def attention(pair):
                for qc in range(QC):
                    oA = pp_o.tile([128, 512], F32, tag="o")
                    oB = pp_o.tile([128, 512], F32, tag="o")
                    qsl = slice(qc * 512, (qc + 1) * 512)
                    for kt in range(TT):
                        ksl = slice(kt * 128, (kt + 1) * 128)
                        sp = pp_score.tile([128, 1024], F32, tag="s")
                        nc.tensor.matmul(sp[:, 0:512], khat[pair][0:64, ksl],
                                         qhat[pair][0:64, qsl],
                                         start=True, stop=True)
                        nc.tensor.matmul(sp[:, 512:1024], khat[pair][64:128, ksl],
                                         qhat[pair][64:128, qsl],
                                         start=True, stop=True)
                        if _dve_exp_tile(pair, qc, kt):
                            pi = psbi.tile([128, 1024], I16, tag="pi")
                            with nc.allow_low_precision(reason="schraudolph exp"):
                                nc.vector.tensor_scalar(pi[:], sp[:], SCHRA_A,
                                                        SCHRA_B, mult, add)
                            pb = pi[:].bitcast(BF16)
                        else:
                            pt = psb.tile([128, 1024], BF16, tag="p")
                            with nc.allow_low_precision(reason="bf16 probs"):
                                nc.scalar.activation(pt[:], sp[:], Exp)
                            pb = pt[:]
                        nc.tensor.matmul(oA[0:65, :], vp[pair][:, kt, 0, :],
                                         pb[:, 0:512],
                                         start=(kt == 0), stop=(kt == TT - 1))
                        nc.tensor.matmul(oB[0:65, :], vp[pair][:, kt, 1, :],
                                         pb[:, 512:1024],
                                         start=(kt == 0), stop=(kt == TT - 1))
                    stA = stgp.tile([65, 512], BF16, tag="stg")
                    stB = stgp.tile([65, 512], BF16, tag="stg")
                    with nc.allow_low_precision(reason="bf16 o evict"):
                        nc.scalar.copy(stA[:], oA[0:65, :])
                        nc.vector.tensor_copy(out=stB[:], in_=oB[0:65, :])
                    nc.sync.dma_start(out=ohp[pair][0:64, qsl], in_=stA[0:64, :])
                    nc.gpsimd.dma_start(
                        out=dn16[pair * 8 + qc:pair * 8 + qc + 1, :],
                        in_=stA[64:65, :])
                    nc.sync.dma_start(out=ohp[pair][64:128, qsl], in_=stB[0:64, :])
                    nc.gpsimd.dma_start(
                        out=dn16[pair * 8 + 4 + qc:pair * 8 + 4 + qc + 1, :],
                        in_=stB[64:65, :])

            # ---- normalize + output projection ----
            def output():
                with nc.allow_low_precision(reason="bf16 recip"):
                    nc.vector.reciprocal(dnr16[:], dn16[:])
                dnrd = drp.tile([16, 512], BF16, tag="dnrd", name="dnrd")
                nc.gpsimd.dma_start(out=dnrd[:], in_=dnr16[:])
                for pair in range(NPAIR):
                    for h in range(2):
                        for qc in range(QC):
                            r = pair * 8 + h * 4 + qc
                            row = dnrd[r:r + 1, :]
                            brd = bass.AP(tensor=row.tensor, offset=row.offset,
                                          ap=[[0, 64]] + list(row.ap[1:]))
                            nc.gpsimd.dma_start(
                                out=rb[pair][64 * h:64 * h + 64,
                                             qc * 512:(qc + 1) * 512],
                                in_=brd)
                    with nc.allow_low_precision(reason="bf16 ohr"):
                        nc.vector.tensor_mul(ohr[pair][:], ohp[pair][:], rb[pair][:])
                for et in range(CT):
                    for qc in range(QC):
                        po = pp_proj.tile([128, 512], F32, tag="proj")
                        for pair in range(NPAIR):
                            nc.tensor.matmul(
                                po[:], wo_sb[pair][:, et * 128:(et + 1) * 128],
                                ohr[pair][:, qc * 512:(qc + 1) * 512],
                                start=(pair == 0), stop=(pair == NPAIR - 1))
                        osb = osbp.tile([128, 512], BF16, tag="osb")
                        with nc.allow_low_precision(reason="bf16 out"):
                            if (et + qc) % 2 == 0:
                                nc.vector.tensor_copy(out=osb[:], in_=po[:])
                            else:
                                nc.scalar.copy(osb[:], po[:])
                        nc.sync.dma_start(
                            out=outT[et * 128:(et + 1) * 128,
                                     qc * 512:(qc + 1) * 512],
                            in_=osb[:])

            vproj()
            proj_norm(0, wq, wqc, qhat[0], "q")
            proj_norm(0, wk, wkc, khat[0], "k")
            attention(0)
            proj_norm(1, wq, wqc, qhat[1], "q")
            proj_norm(1, wk, wkc, khat[1], "k")
            attention(1)
            output()

    nc.compile()
    return nc


def _get_compiled():
    global _COMPILED
    if _COMPILED is None:
        _COMPILED = _build()
    return _COMPILED


def _make_consts(q_norm_w, k_norm_w):
    bd2 = np.zeros((128, 2), np.float32)
    bd2[0:64, 0] = 1.0
    bd2[64:128, 1] = 1.0
    wqc = np.concatenate([q_norm_w, q_norm_w]).reshape(128, 1).astype(np.float32)
    wkc = np.concatenate([k_norm_w, k_norm_w]).reshape(128, 1).astype(np.float32)
    return bd2, wqc, wkc


def make_in_maps(x, wq, wk, wv, wo, q_norm_w, k_norm_w):
    import ml_dtypes

    cast = lambda a: np.ascontiguousarray(a).astype(ml_dtypes.bfloat16)
    bd2, wqc, wkc = _make_consts(q_norm_w, k_norm_w)
    in_maps = []
    for c in range(N_CORES):
        b = c // 4
        hs = HPC * (c % 4)
        # head split in reference is strided: head h uses columns d*H + h
        perm = ((hs + np.arange(HPC))[:, None]
                + H * np.arange(HD)[None, :]).reshape(-1)
        in_maps.append({
            "xbT": cast(x[b].T),
            "wq_s": cast(wq[:, perm]),
            "wk_s": cast(wk[:, perm]),
            "wv_s": cast(wv[:, perm]),
            "wo_s": cast(wo[hs * HD:(hs + HPC) * HD, :]),
            "bd2": cast(bd2),
            "wqc": wqc, "wkc": wkc,
        })
    return in_maps


def kernel(x, wq, wk, wv, wo, q_norm_w, k_norm_w):
    from concourse.bass_utils import run_bass_kernel_spmd

    global LAST_EXEC_NS
    if os.environ.get("BASS_TRACE"):
        _install_ntff_shim()

    x = np.asarray(x, dtype=np.float32)
    wq = np.asarray(wq, dtype=np.float32)
    wk = np.asarray(wk, dtype=np.float32)
    wv = np.asarray(wv, dtype=np.float32)
    wo = np.asarray(wo, dtype=np.float32)
    q_norm_w = np.asarray(q_norm_w, dtype=np.float32)
    k_norm_w = np.asarray(k_norm_w, dtype=np.float32)

    nc = _get_compiled()
    in_maps = make_in_maps(x, wq, wk, wv, wo, q_norm_w, k_norm_w)

    res = run_bass_kernel_spmd(nc, in_maps, core_ids=list(range(N_CORES)),
                               trace=bool(os.environ.get("BASS_TRACE")),
                               tmpdir=os.environ.get("BASS_TRACE_DIR"))
    LAST_EXEC_NS = res.exec_time_ns

    out = np.empty((B, T, D), dtype=np.float32)
    for b in range(B):
        acc = res.results[4 * b]["outT"].astype(np.float32)
        for c in range(4 * b + 1, 4 * b + 4):
            acc = acc + res.results[c]["outT"].astype(np.float32)
        out[b] = acc.T
    return out


# revision 16
# speedup vs baseline: 1.1148x; 1.1148x over previous
"""Multi-head attention (QKV proj + per-head RMSNorm + softmax attention +
output proj) for Trainium2, distributed over 8 NeuronCores.

Sharding: batch (2) x head-groups (4 heads per core), as one software-pipelined
pass (no phase barrier) so TensorE projection/output work overlaps the
ACT/DVE exp work of the attention phase.

Key structure per core (b = core//4, 4 heads = 2 pairs):
- X^T loaded pretransposed [D, T]; Q^T/K^T projections per pair in a packed
  [128 = 2 heads x 64d, T] layout (head A rows 0:64, head B rows 64:128).
- V projected directly in [token, d] layout (stationary = X^T tile, moving =
  Wv) into [V|1] staging tiles, so no PE transposes are needed and the 65th
  column accumulates softmax denominators during the AV matmul.
- RMSNorm: sum-of-squares via a [128,2] ones matmul, rstd = exp(-0.5*ln(.))
  on ACT in packed [8,512] tiles (thin [2,512] chunks are repacked via tiny
  DMAs), broadcast back via DRAM round-trip DMAs.  The softmax 1/8 scale and
  k_norm weight fold into khat; q_norm weight folds into qhat.
- Scores S^T = K^T.T @ Q^T run as TWO CONCURRENT row-tiled matmuls
  (tile_position (0,0) / (64,0), K=64 each) writing the two halves of one
  [128, 1024] PSUM tile -> 2x score throughput vs zero-padded K=128.
- exp over scores is split between ScalarE (table exp, bf16 out) and VectorE
  (Schraudolph bit-trick: i16 = s*A + B, bitcast to bf16) on a tunable
  fraction of key-tiles, so neither engine gates the TensorE pipeline.
- Attention output O^T accumulates in PSUM [65, 512] per (head, query-chunk);
  row 64 is the denominator.  Evictions go bf16 to SBUF staging, then DMA
  assembles ohp [128, T] per pair and a packed dn [16, 512] tile; ONE DVE
  reciprocal + DRAM broadcast gives 1/dn; output projection is a plain
  2-pair accumulated matmul, partial outputs summed on host.
"""

import os
import sys

for _p in ("/opt/trn_rl_repo",):
    if _p not in sys.path:
        sys.path.insert(0, _p)

import numpy as np

B = 2
T = 2048
D = 1024
H = 16
HD = 64
HPC = 4          # heads per core
NPAIR = 2        # head pairs per core
N_CORES = 8
EPS = 1e-5
TT = T // 128    # 16 key tiles
CT = D // 128    # 8 contraction tiles
QC = T // 512    # 4 query chunks

# Schraudolph bf16 exp: bits = trunc(s * A + B) as int16, bitcast to bf16.
SCHRA_A = 184.66496280360212     # 128 / ln(2)
SCHRA_B = 16249.05               # (127 - 6.95) * 128 rounded; tuned offline

_COMPILED = None
LAST_EXEC_NS = None


def _dve_exp_tile(pair, qc, kt):
    """Which (pair, qc, kt) exp tiles run on VectorE (Schraudolph)."""
    if pair == 0:
        return kt % 8 == 7
    return kt % 2 == 1


def _install_ntff_shim():
    """antenv.axon_hooks is missing in this image; provide it so that
    BASS_TRACE=1 profiling works (mirrors trn_boot's ctypes hook)."""
    import contextlib
    import ctypes
    import types

    if "antenv.axon_hooks" in sys.modules:
        return
    so_path = "/opt/axon/libaxon_pjrt.so"
    if not os.path.exists(so_path):
        return
    lib = ctypes.CDLL(so_path)
    if not hasattr(lib, "axon_start_nrt_profile"):
        return
    lib.axon_start_nrt_profile.argtypes = [ctypes.POINTER(ctypes.c_int64), ctypes.c_size_t]
    lib.axon_start_nrt_profile.restype = ctypes.c_int64
    lib.axon_stop_nrt_profile.argtypes = [ctypes.c_char_p]
    lib.axon_stop_nrt_profile.restype = ctypes.c_int64

    @contextlib.contextmanager
    def _hook(output_dir, device_ids):
        import jax

        jax.devices()
        if device_ids:
            ids = (ctypes.c_int64 * len(device_ids))(*device_ids)
            rc = lib.axon_start_nrt_profile(ids, len(device_ids))
        else:
            rc = lib.axon_start_nrt_profile(None, 0)
        if rc != 0:
            raise RuntimeError(f"axon_start_nrt_profile rc={rc}")
        try:
            yield
        finally:
            n = lib.axon_stop_nrt_profile(str(output_dir).encode())
            print(f"profile: {n} file(s) written to {output_dir}", file=sys.stderr)

    mod = types.ModuleType("antenv.axon_hooks")
    mod._hook = _hook
    mod.get_axon_ntff_profile_hook = lambda: mod._hook
    mod.set_axon_ntff_profile_hook = lambda h: setattr(mod, "_hook", h)
    sys.modules["antenv.axon_hooks"] = mod
    try:
        import antenv

        antenv.axon_hooks = mod
    except ImportError:
        pass


def _build():
    import concourse.bass as bass
    import concourse.tile as tile
    from concourse import bacc, mybir
    from contextlib import ExitStack

    F32 = mybir.dt.float32
    BF16 = mybir.dt.bfloat16
    I16 = mybir.dt.int16
    Exp = mybir.ActivationFunctionType.Exp
    Square = mybir.ActivationFunctionType.Square
    Log = mybir.ActivationFunctionType.Ln if hasattr(
        mybir.ActivationFunctionType, "Ln") else mybir.ActivationFunctionType.Log
    mult = mybir.AluOpType.mult
    add = mybir.AluOpType.add
    LN8 = float(np.log(0.125))

    nc = bacc.Bacc("TRN2", target_bir_lowering=False, debug=False, num_devices=N_CORES)

    xbT_d = nc.dram_tensor("xbT", (D, T), BF16, kind="ExternalInput").ap()
    wq_d = nc.dram_tensor("wq_s", (D, HPC * HD), BF16, kind="ExternalInput").ap()
    wk_d = nc.dram_tensor("wk_s", (D, HPC * HD), BF16, kind="ExternalInput").ap()
    wv_d = nc.dram_tensor("wv_s", (D, HPC * HD), BF16, kind="ExternalInput").ap()
    wo_d = nc.dram_tensor("wo_s", (HPC * HD, D), BF16, kind="ExternalInput").ap()
    bd2_d = nc.dram_tensor("bd2", (128, 2), BF16, kind="ExternalInput").ap()
    wqc_d = nc.dram_tensor("wqc", (128, 1), F32, kind="ExternalInput").ap()
    wkc_d = nc.dram_tensor("wkc", (128, 1), F32, kind="ExternalInput").ap()
    outT = nc.dram_tensor("outT", (D, T), BF16, kind="ExternalOutput").ap()

    with tile.TileContext(nc) as tc:
        with ExitStack() as top:
            consts = top.enter_context(tc.tile_pool(name="consts", bufs=1))
            xtp = top.enter_context(tc.tile_pool(name="xT", bufs=1))
            wpool = top.enter_context(tc.tile_pool(name="w", bufs=1))
            qkpool = top.enter_context(tc.tile_pool(name="qk", bufs=1))
            vppool = top.enter_context(tc.tile_pool(name="vp", bufs=1))
            stage = top.enter_context(tc.tile_pool(name="stage", bufs=3))
            sqp = top.enter_context(tc.tile_pool(name="sq", bufs=5))
            mstp = top.enter_context(tc.tile_pool(name="mst", bufs=2))
            msp = top.enter_context(tc.tile_pool(name="ms", bufs=1))
            rwp = top.enter_context(tc.tile_pool(name="rw", bufs=2))
            psb = top.enter_context(tc.tile_pool(name="p", bufs=3))
            psbi = top.enter_context(tc.tile_pool(name="pi", bufs=2))
            stgp = top.enter_context(tc.tile_pool(name="stg", bufs=3))
            ohpp = top.enter_context(tc.tile_pool(name="ohp", bufs=1))
            dnp = top.enter_context(tc.tile_pool(name="dn", bufs=1))
            rbp = top.enter_context(tc.tile_pool(name="rb", bufs=1))
            ohrp = top.enter_context(tc.tile_pool(name="ohr", bufs=1))
            osbp = top.enter_context(tc.tile_pool(name="osb", bufs=2))
            drp = top.enter_context(tc.tile_pool(name="dr", bufs=1, space="DRAM"))
            pp_score = top.enter_context(
                tc.tile_pool(name="ppscore", bufs=2, space="PSUM"))
            pp_o = top.enter_context(tc.tile_pool(name="ppo", bufs=2, space="PSUM"))
            pp_proj = top.enter_context(
                tc.tile_pool(name="ppproj", bufs=2, space="PSUM"))

            # ---- consts + weight/activation loads ----
            bd2 = consts.tile([128, 2], BF16, tag="bd2")
            nc.sync.dma_start(out=bd2[:], in_=bd2_d)
            wqc = consts.tile([128, 1], F32, tag="wqc")
            nc.sync.dma_start(out=wqc[:], in_=wqc_d)
            wkc = consts.tile([128, 1], F32, tag="wkc")
            nc.sync.dma_start(out=wkc[:], in_=wkc_d)
            epsc = consts.tile([128, 1], F32, tag="epsc")
            nc.vector.memset(epsc[:], EPS)
            ln8c = consts.tile([128, 1], F32, tag="ln8c")
            nc.vector.memset(ln8c[:], LN8)

            xT = []
            for c in range(CT):
                t = xtp.tile([128, T], BF16, tag=f"xT{c}", name=f"xT{c}")
                nc.sync.dma_start(out=t[:], in_=xbT_d[c * 128:(c + 1) * 128, :])
                xT.append(t)

            def load_w(dram, nm, eng):
                ws = []
                for c in range(CT):
                    t = wpool.tile([128, HPC * HD], BF16, tag=f"w{nm}{c}",
                                   name=f"w{nm}{c}")
                    eng.dma_start(out=t[:], in_=dram[c * 128:(c + 1) * 128, :])
                    ws.append(t)
                return ws

            wq = load_w(wq_d, "q", nc.scalar)
            wk = load_w(wk_d, "k", nc.scalar)
            wv = load_w(wv_d, "v", nc.gpsimd)
            wo_sb = []
            for p in range(NPAIR):
                t = wpool.tile([128, D], BF16, tag=f"wo{p}", name=f"wo{p}")
                nc.gpsimd.dma_start(out=t[:], in_=wo_d[p * 128:(p + 1) * 128, :])
                wo_sb.append(t)

            # persistent data tiles
            vp = [vppool.tile([128, TT, 2, 65], BF16, tag=f"vp{p}", name=f"vp{p}")
                  for p in range(NPAIR)]
            for p in range(NPAIR):
                nc.vector.memset(vp[p][:, :, :, 64:65], 1.0)
            qhat = [qkpool.tile([128, T], BF16, tag=f"qh{p}", name=f"qhat{p}")
                    for p in range(NPAIR)]
            khat = [qkpool.tile([128, T], BF16, tag=f"kh{p}", name=f"khat{p}")
                    for p in range(NPAIR)]
            ohp = [ohpp.tile([128, T], BF16, tag=f"ohp{p}", name=f"ohp{p}")
                   for p in range(NPAIR)]
            dn8 = [dnp.tile([8, 512], BF16, tag=f"dn8_{p}", name=f"dn8_{p}")
                   for p in range(NPAIR)]
            rb = [rbp.tile([128, T], BF16, tag=f"rb{p}", name=f"rb{p}")
                  for p in range(NPAIR)]
            ohr = [ohrp.tile([128, T], BF16, tag=f"ohr{p}", name=f"ohr{p}")
                   for p in range(NPAIR)]

            # ---- V: direct [token, d] projection into [V|1] staging ----
            def vproj():
                for kt in range(TT):
                    pv = pp_proj.tile([128, HPC * HD], F32, tag="proj")
                    for ct in range(CT):
                        nc.tensor.matmul(
                            pv[:], xT[ct][:, kt * 128:(kt + 1) * 128], wv[ct][:],
                            start=(ct == 0), stop=(ct == CT - 1))
                    for pair in range(NPAIR):
                        src = pv[:, pair * 128:(pair + 1) * 128].rearrange(
                            "p (h d) -> p h d", h=2)
                        with nc.allow_low_precision(reason="bf16 v staging"):
                            if (kt + pair) % 2 == 0:
                                nc.scalar.copy(vp[pair][:, kt, :, 0:64], src)
                            else:
                                nc.vector.tensor_copy(
                                    out=vp[pair][:, kt, :, 0:64], in_=src)

            # ---- Q/K projection + RMSNorm chain for one (pair, kind) ----
            def proj_norm(pair, wt, wcol, dest, kind):
                stg = stage.tile([128, T], BF16, tag="qkstage",
                                 name=f"stg{kind}{pair}")
                ms8 = msp.tile([8, 512], F32, tag=f"ms{kind}{pair}",
                               name=f"ms{kind}{pair}")
                q2s = []
                for qc in range(QC):
                    pq = pp_proj.tile([128, 512], F32, tag="proj")
                    for ct in range(CT):
                        nc.tensor.matmul(
                            pq[:], wt[ct][:, pair * 128:(pair + 1) * 128],
                            xT[ct][:, qc * 512:(qc + 1) * 512],
                            start=(ct == 0), stop=(ct == CT - 1))
                    sl = slice(qc * 512, (qc + 1) * 512)
                    with nc.allow_low_precision(reason="bf16 evict"):
                        nc.vector.tensor_scalar(stg[:, sl], pq[:], wcol[:], None,
                                                mult)
                    q2 = sqp.tile([128, 512], BF16, tag="q2")
                    with nc.allow_low_precision(reason="bf16 squares"):
                        nc.scalar.activation(q2[:], pq[:], Square)
                    q2s.append(q2)
                # sumsq matmuls deferred so the PE never waits on ACT squares
                for qc in range(QC):
                    msps = pp_proj.tile([128, 512], F32, tag="proj")
                    nc.tensor.matmul(msps[0:2, :], bd2[:], q2s[qc][:],
                                     start=True, stop=True)
                    mstmp = mstp.tile([2, 512], F32, tag="mstmp")
                    nc.vector.tensor_copy(out=mstmp[:], in_=msps[0:2, :])
                    nc.gpsimd.dma_start(out=ms8[2 * qc:2 * qc + 2, :], in_=mstmp[:])
                ms8ln = msp.tile([8, 512], F32, tag=f"msln{kind}{pair}",
                                 name=f"msln{kind}{pair}")
                nc.scalar.activation(ms8ln[:], ms8[:], Log,
                                     scale=1.0 / HD, bias=epsc[0:8, :])
                r8 = msp.tile([8, 512], BF16, tag=f"r8{kind}{pair}",
                              name=f"r8{kind}{pair}")
                with nc.allow_low_precision(reason="bf16 rstd"):
                    if kind == "k":
                        nc.scalar.activation(r8[:], ms8ln[:], Exp, scale=-0.5,
                                             bias=ln8c[0:8, :])
                    else:
                        nc.scalar.activation(r8[:], ms8ln[:], Exp, scale=-0.5)
                r8d = drp.tile([8, 512], BF16, tag=f"r8d{kind}{pair}",
                               name=f"r8d{kind}{pair}")
                nc.gpsimd.dma_start(out=r8d[:], in_=r8[:])
                rw = rwp.tile([128, T], BF16, tag="rw")
                base = r8d[:]
                for h in range(2):
                    # rows h, h+2, h+4, h+6 broadcast to 64 partitions, one DMA
                    brd = bass.AP(tensor=base.tensor, offset=base.offset + h * 512,
                                  ap=[[0, 64], [1024, QC], [1, 512]])
                    nc.gpsimd.dma_start(out=rw[64 * h:64 * h + 64, :], in_=brd)
                with nc.allow_low_precision(reason="bf16 norm"):
                    nc.vector.tensor_mul(dest[:], stg[:], rw[:])

            # ---- attention for one pair ----
            def attention(pair):
                for qc in range(QC):
                    oA = pp_o.tile([128, 512], F32, tag="o")
                    oB = pp_o.tile([128, 512], F32, tag="o")
                    qsl = slice(qc * 512, (qc + 1) * 512)
                    for kt in range(TT):
                        ksl = slice(kt * 128, (kt + 1) * 128)
                        sp = pp_score.tile([128, 1024], F32, tag="s")
                        nc.tensor.matmul(sp[:, 0:512], khat[pair][0:64, ksl],
                                         qhat[pair][0:64, qsl],
                                         start=True, stop=True)
                        nc.tensor.matmul(sp[:, 512:1024], khat[pair][64:128, ksl],
                                         qhat[pair][64:128, qsl],
                                         start=True, stop=True)
                        if _dve_exp_tile(pair, qc, kt):
                            pi = psbi.tile([128, 1024], I16, tag="pi")
                            with nc.allow_low_precision(reason="schraudolph exp"):
                                nc.vector.tensor_scalar(pi[:], sp[:], SCHRA_A,
                                                        SCHRA_B, mult, add)
                            pb = pi[:].bitcast(BF16)
                        else:
                            pt = psb.tile([128, 1024], BF16, tag="p")
                            with nc.allow_low_precision(reason="bf16 probs"):
                                nc.scalar.activation(pt[:], sp[:], Exp)
                            pb = pt[:]
                        nc.tensor.matmul(oA[0:65, :], vp[pair][:, kt, 0, :],
                                         pb[:, 0:512],
                                         start=(kt == 0), stop=(kt == TT - 1))
                        nc.tensor.matmul(oB[0:65, :], vp[pair][:, kt, 1, :],
                                         pb[:, 512:1024],
                                         start=(kt == 0), stop=(kt == TT - 1))
                    stA = stgp.tile([65, 512], BF16, tag="stg")
                    stB = stgp.tile([65, 512], BF16, tag="stg")
                    with nc.allow_low_precision(reason="bf16 o evict"):
                        nc.scalar.copy(stA[:], oA[0:65, :])
                        nc.vector.tensor_copy(out=stB[:], in_=oB[0:65, :])
                    nc.sync.dma_start(out=ohp[pair][0:64, qsl], in_=stA[0:64, :])
                    nc.gpsimd.dma_start(out=dn8[pair][qc:qc + 1, :],
                                        in_=stA[64:65, :])
                    nc.sync.dma_start(out=ohp[pair][64:128, qsl], in_=stB[0:64, :])
                    nc.gpsimd.dma_start(out=dn8[pair][4 + qc:4 + qc + 1, :],
                                        in_=stB[64:65, :])

            # ---- per-pair denominator chain: 1/dn, broadcast, normalize ----
            def dn_chain(pair):
                dnf = dnp.tile([8, 512], F32, tag=f"dnf{pair}", name=f"dnf{pair}")
                nc.vector.tensor_copy(out=dnf[:], in_=dn8[pair][:])
                dnrf = dnp.tile([8, 512], F32, tag=f"dnrf{pair}",
                                name=f"dnrf{pair}")
                nc.vector.reciprocal_approx_fast(out=dnrf[:], in_=dnf[:])
                dnr8 = dnp.tile([8, 512], BF16, tag=f"dnr8{pair}",
                                name=f"dnr8{pair}")
                with nc.allow_low_precision(reason="bf16 recip"):
                    nc.vector.tensor_copy(out=dnr8[:], in_=dnrf[:])
                dnrd = drp.tile([8, 512], BF16, tag=f"dnrd{pair}",
                                name=f"dnrd{pair}")
                nc.gpsimd.dma_start(out=dnrd[:], in_=dnr8[:])
                base = dnrd[:]
                for h in range(2):
                    brd = bass.AP(tensor=base.tensor,
                                  offset=base.offset + h * 4 * 512,
                                  ap=[[0, 64], [512, QC], [1, 512]])
                    nc.gpsimd.dma_start(out=rb[pair][64 * h:64 * h + 64, :],
                                        in_=brd)
                with nc.allow_low_precision(reason="bf16 ohr"):
                    nc.vector.tensor_mul(ohr[pair][:], ohp[pair][:], rb[pair][:])

            # ---- output projection ----
            def output():
                for et in range(CT):
                    osb = osbp.tile([128, T], BF16, tag="osb")
                    for qc in range(QC):
                        po = pp_proj.tile([128, 512], F32, tag="proj")
                        for pair in range(NPAIR):
                            nc.tensor.matmul(
                                po[:], wo_sb[pair][:, et * 128:(et + 1) * 128],
                                ohr[pair][:, qc * 512:(qc + 1) * 512],
                                start=(pair == 0), stop=(pair == NPAIR - 1))
                        osl = slice(qc * 512, (qc + 1) * 512)
                        with nc.allow_low_precision(reason="bf16 out"):
                            if qc % 2 == 0:
                                nc.vector.tensor_copy(out=osb[:, osl], in_=po[:])
                            else:
                                nc.scalar.copy(osb[:, osl], po[:])
                    (nc.sync if et % 2 == 0 else nc.gpsimd).dma_start(
                        out=outT[et * 128:(et + 1) * 128, :], in_=osb[:])

            proj_norm(0, wq, wqc, qhat[0], "q")
            proj_norm(0, wk, wkc, khat[0], "k")
            vproj()
            attention(0)
            dn_chain(0)
            proj_norm(1, wq, wqc, qhat[1], "q")
            proj_norm(1, wk, wkc, khat[1], "k")
            attention(1)
            dn_chain(1)
            output()

    nc.compile()
    return nc


def _get_compiled():
    global _COMPILED
    if _COMPILED is None:
        _COMPILED = _build()
    return _COMPILED


def _make_consts(q_norm_w, k_norm_w):
    bd2 = np.zeros((128, 2), np.float32)
    bd2[0:64, 0] = 1.0
    bd2[64:128, 1] = 1.0
    wqc = np.concatenate([q_norm_w, q_norm_w]).reshape(128, 1).astype(np.float32)
    wkc = np.concatenate([k_norm_w, k_norm_w]).reshape(128, 1).astype(np.float32)
    return bd2, wqc, wkc


def make_in_maps(x, wq, wk, wv, wo, q_norm_w, k_norm_w):
    import ml_dtypes

    cast = lambda a: np.ascontiguousarray(a).astype(ml_dtypes.bfloat16)
    bd2, wqc, wkc = _make_consts(q_norm_w, k_norm_w)
    in_maps = []
    for c in range(N_CORES):
        b = c // 4
        hs = HPC * (c % 4)
        # head split in reference is strided: head h uses columns d*H + h
        perm = ((hs + np.arange(HPC))[:, None]
                + H * np.arange(HD)[None, :]).reshape(-1)
        in_maps.append({
            "xbT": cast(x[b].T),
            "wq_s": cast(wq[:, perm]),
            "wk_s": cast(wk[:, perm]),
            "wv_s": cast(wv[:, perm]),
            "wo_s": cast(wo[hs * HD:(hs + HPC) * HD, :]),
            "bd2": cast(bd2),
            "wqc": wqc, "wkc": wkc,
        })
    return in_maps


def kernel(x, wq, wk, wv, wo, q_norm_w, k_norm_w):
    from concourse.bass_utils import run_bass_kernel_spmd

    global LAST_EXEC_NS
    if os.environ.get("BASS_TRACE"):
        _install_ntff_shim()

    x = np.asarray(x, dtype=np.float32)
    wq = np.asarray(wq, dtype=np.float32)
    wk = np.asarray(wk, dtype=np.float32)
    wv = np.asarray(wv, dtype=np.float32)
    wo = np.asarray(wo, dtype=np.float32)
    q_norm_w = np.asarray(q_norm_w, dtype=np.float32)
    k_norm_w = np.asarray(k_norm_w, dtype=np.float32)

    nc = _get_compiled()
    in_maps = make_in_maps(x, wq, wk, wv, wo, q_norm_w, k_norm_w)

    res = run_bass_kernel_spmd(nc, in_maps, core_ids=list(range(N_CORES)),
                               trace=bool(os.environ.get("BASS_TRACE")),
                               tmpdir=os.environ.get("BASS_TRACE_DIR"))
    LAST_EXEC_NS = res.exec_time_ns

    out = np.empty((B, T, D), dtype=np.float32)
    for b in range(B):
        acc = res.results[4 * b]["outT"].astype(np.float32)
        for c in range(4 * b + 1, 4 * b + 4):
            acc = acc + res.results[c]["outT"].astype(np.float32)
        out[b] = acc.T
    return out
